# revision 1
# baseline (speedup 1.0000x reference)
"""Chebyshev (L-inf) pairwise distance matrix on 8 TRN2 NeuronCores.

reference: out[i, j] = max_d |embed1[i, d] - embed2[j, d]|
  embed1: [4096, 32] f32, embed2: [4096, 32] f32, out: [4096, 4096] f32

Sharding: 8 cores = 4 i-quarters x 2 j-halves. Each core computes the
[2048 j, 1024 i] transposed block of the output.

Per-core layout: partition axis = j (16 blocks of 128), free axis = i (1024).
For each j-block and each d, the absdiff |e1[i,d] - e2[j,d]| is computed as
either an ACT activation-Abs (bias = -e2[j,d] per partition) or a DVE
tensor_scalar subtract (4x bf16 perf mode) whose sign bit is then cleared by
a single wide bitwise-and on the uint16 view. The tensor operand is e1's
column d broadcast across the 128 partitions (host-prepped, DMA'd once);
the -e2 bias table is negated on-chip from the e2 load.
The max-reduction over d is an in-place wide max tensor_tensor tree on DVE
(2x bf16 mode). Output is bf16, upcast on host. GPSIMD/PE are unusable here:
walrus rejects TensorTensor/TensorScalar on Pool for core v3, and abs_max
is not encodable at all.
"""

import sys

if "/opt/trn_rl_repo" not in sys.path:
    sys.path.insert(0, "/opt/trn_rl_repo")

from contextlib import ExitStack

import ml_dtypes
import numpy as np

import concourse.bacc as bacc
import concourse.bass as bass
import concourse.tile as tile
from concourse import mybir

BF16 = ml_dtypes.bfloat16

N = 4096          # rows of embed1 (= rows of embed2)
D = 32            # feature dim
N_CORES = 8
N_IQ = 4          # i split (embed1 rows)
N_JH = 2          # j split (embed2 rows)
I_PER = N // N_IQ       # 1024 per core
J_PER = N // N_JH       # 2048 per core
JB = J_PER // 128       # 16 j-blocks per core
U = I_PER               # free-dim elements per d-slot

# d ownership for the absdiff stage: ACT does d[0:23], DVE d[23:32].
N_ACT = 23
N_DVE = 9
assert N_ACT + N_DVE == D
# e1r_act arrives in chunks so the first ACT ops don't wait on the full 6 MB;
# tiny first chunk => ACT starts after ~0.5 MB of DMA instead of 2 MB.
ACT_CHUNKS = (2, 7, 7, 7)
assert sum(ACT_CHUNKS) == N_ACT

_nc_cache = None


def _build_nc():
    nc = bacc.Bacc(
        trn_type="TRN2",
        target_bir_lowering=False,
        debug=False,
        num_devices=N_CORES,
    )

    dt_bf16 = mybir.dt.bfloat16
    dt_u16 = mybir.dt.uint16
    dt_f32 = mybir.dt.float32

    # e1 slab transposed to d-major and broadcast across 128 partitions
    # (host side), split by absdiff owner.
    e1r_act = nc.declare_dram_parameter("e1r_act", [128, N_ACT * U], dt_bf16, isOutput=False)
    e1r_dve = nc.declare_dram_parameter("e1r_dve", [128, N_DVE * U], dt_bf16, isOutput=False)
    # e2 j-half slab [J_PER, 32] f32 (negated on-chip for the ACT bias).
    e2b = nc.declare_dram_parameter("e2b", [J_PER, D], dt_f32, isOutput=False)
    out = nc.declare_dram_parameter("out", [J_PER, I_PER], dt_bf16, isOutput=True)

    vmax = mybir.AluOpType.max
    sub = mybir.AluOpType.subtract
    band = mybir.AluOpType.bitwise_and

    with tile.TileContext(nc) as tc, ExitStack() as ctx:
        p_e1 = ctx.enter_context(tc.tile_pool(name="e1", bufs=1))
        p_e2 = ctx.enter_context(tc.tile_pool(name="e2", bufs=1))
        p_act = ctx.enter_context(tc.tile_pool(name="ract", bufs=2))
        p_dve = ctx.enter_context(tc.tile_pool(name="rdve", bufs=1))
        p_out = ctx.enter_context(tc.tile_pool(name="out", bufs=2))

        # --- one-time loads, smallest-first so both engines start early ---
        t_e2 = p_e2.tile([128, JB * D], dt_f32, tag="e2")
        t_e2n = p_e2.tile([128, JB * D], dt_f32, tag="e2n")
        e2_src = e2b[:, :].rearrange("(jb p) d -> p jb d", p=128)
        nc.sync.dma_start(t_e2[:].rearrange("p (jb d) -> p jb d", d=D), e2_src)
        # ACT bias wants -e2; negate on-chip (tiny op) instead of a 2nd DMA
        nc.vector.tensor_scalar(t_e2n[:], t_e2[:], -1.0, None,
                                op0=mybir.AluOpType.mult)

        # first ACT chunk (2 slabs) lands fast so ACT starts ~4us in; the whole
        # DVE region (9 slabs) next; remaining ACT chunks stream in behind
        t_e1a_chunks = []
        off_a = ACT_CHUNKS[0]
        t0 = p_e1.tile([128, ACT_CHUNKS[0] * U], dt_bf16, tag="e1a0")
        nc.sync.dma_start(t0[:], e1r_act[:, :off_a * U])
        t_e1a_chunks.append((0, ACT_CHUNKS[0], t0))
        t_e1d = p_e1.tile([128, N_DVE * U], dt_bf16, tag="e1d")
        nc.sync.dma_start(t_e1d[:], e1r_dve[:, :])
        for ci, csz in enumerate(ACT_CHUNKS[1:], 1):
            t = p_e1.tile([128, csz * U], dt_bf16, tag=f"e1a{ci}")
            nc.sync.dma_start(t[:], e1r_act[:, off_a * U:(off_a + csz) * U])
            t_e1a_chunks.append((off_a, csz, t))
            off_a += csz

        def emit_block(jb, i_lo, w, seq_ract=False):
            """absdiff + reduce + store for j-block jb, i-range [i_lo, i_lo+w)."""
            r_a = p_act.tile([128, N_ACT * w], dt_bf16, tag="ract")
            r_d = p_dve.tile([128, N_DVE * w], dt_bf16, tag="rdve")

            # --- absdiff stage ---
            # DVE's independent work first (keeps DVE busy while ACT runs)
            for k in range(N_DVE):
                d = N_ACT + k
                # raw diff; abs happens in the wide sign-clear below
                nc.vector.tensor_scalar(
                    r_d[:, k * w:(k + 1) * w],
                    t_e1d[:, k * U + i_lo:k * U + i_lo + w],
                    t_e2[:, jb * D + d: jb * D + d + 1],
                    None,
                    op0=sub,
                )
            # clear bf16 sign bits of the whole DVE region in one wide op
            r_d_u16 = r_d[:].bitcast(dt_u16)
            nc.vector.tensor_scalar(r_d_u16, r_d_u16, 0x7FFF, None, op0=band)
            for off, csz, t in t_e1a_chunks:
                for kk in range(csz):
                    k = off + kk
                    d = k
                    # out = Abs(in * 1.0 + (-e2col))
                    nc.scalar.activation(
                        r_a[:, k * w:(k + 1) * w],
                        t[:, kk * U + i_lo:kk * U + i_lo + w],
                        mybir.ActivationFunctionType.Abs,
                        bias=t_e2n[:, jb * D + d: jb * D + d + 1],
                        scale=1.0,
                    )

            # --- reduction: in-place wide max trees (DVE) ---
            # DVE region: 9 slots -> 4 -> 2 -> 1 (+ ragged 9th)
            nc.vector.tensor_tensor(r_d[:, :4 * w], r_d[:, :4 * w], r_d[:, 4 * w:8 * w], op=vmax)
            nc.vector.tensor_tensor(r_d[:, :2 * w], r_d[:, :2 * w], r_d[:, 2 * w:4 * w], op=vmax)
            nc.vector.tensor_tensor(r_d[:, :w], r_d[:, :w], r_d[:, w:2 * w], op=vmax)
            nc.vector.tensor_tensor(r_d[:, :w], r_d[:, :w], r_d[:, 8 * w:9 * w], op=vmax)
            if seq_ract:
                # last block: narrow sequential accumulation — each max op
                # chases the matching ACT absdiff, so after ACT's final slab
                # only ~1 op remains (shrinks the kernel-tail bubble)
                for k in range(1, N_ACT):
                    nc.vector.tensor_tensor(r_a[:, :w], r_a[:, :w],
                                            r_a[:, k * w:(k + 1) * w], op=vmax)
            else:
                # ACT region: 23 slots -> 8(+7) -> 4 -> 2 -> 1
                nc.vector.tensor_tensor(r_a[:, :8 * w], r_a[:, :8 * w], r_a[:, 8 * w:16 * w], op=vmax)
                nc.vector.tensor_tensor(r_a[:, :7 * w], r_a[:, :7 * w], r_a[:, 16 * w:23 * w], op=vmax)
                nc.vector.tensor_tensor(r_a[:, :4 * w], r_a[:, :4 * w], r_a[:, 4 * w:8 * w], op=vmax)
                nc.vector.tensor_tensor(r_a[:, :2 * w], r_a[:, :2 * w], r_a[:, 2 * w:4 * w], op=vmax)
                nc.vector.tensor_tensor(r_a[:, :w], r_a[:, :w], r_a[:, w:2 * w], op=vmax)

            # --- final merge + store ---
            t_out = p_out.tile([128, w], dt_bf16, tag="out")
            nc.vector.tensor_tensor(t_out[:], r_a[:, :w], r_d[:, :w], op=vmax)

            nc.sync.dma_start(out[jb * 128:(jb + 1) * 128, i_lo:i_lo + w], t_out[:])

        for jb in range(JB):
            emit_block(jb, 0, U)

    nc.finalize()
    return nc


def _get_nc():
    global _nc_cache
    if _nc_cache is None:
        _nc_cache = _build_nc()
    return _nc_cache


def make_in_maps(embed1: np.ndarray, embed2: np.ndarray):
    """Host-side sharding/prep. Returns in_maps for cores 0..7.

    Core c: iq = c % N_IQ, jh = c // N_IQ.
    """
    embed1 = np.asarray(embed1, dtype=np.float32)
    embed2 = np.asarray(embed2, dtype=np.float32)
    in_maps = []
    for c in range(N_CORES):
        iq, jh = c % N_IQ, c // N_IQ
        e1_slab = embed1[iq * I_PER:(iq + 1) * I_PER, :]      # [1024, 32]
        # d-major flatten, bf16, broadcast to 128 partitions
        flat = np.ascontiguousarray(e1_slab.T).reshape(-1).astype(BF16)  # [32*1024]
        rep = np.ascontiguousarray(np.broadcast_to(flat[None, :], (128, D * I_PER)))
        e2_slab = np.ascontiguousarray(embed2[jh * J_PER:(jh + 1) * J_PER, :])  # [2048, 32]
        in_maps.append({
            "e1r_act": np.ascontiguousarray(rep[:, :N_ACT * U]),
            "e1r_dve": np.ascontiguousarray(rep[:, N_ACT * U:]),
            "e2b": e2_slab,
        })
    return in_maps


def assemble(results) -> np.ndarray:
    """results: list of per-core dicts with 'out' [J_PER, I_PER] bf16."""
    full = np.empty((N, N), dtype=np.float32)
    for c in range(N_CORES):
        iq, jh = c % N_IQ, c // N_IQ
        blk = np.asarray(results[c]["out"]).astype(np.float32)  # [2048, 1024]
        full[iq * I_PER:(iq + 1) * I_PER, jh * J_PER:(jh + 1) * J_PER] = blk.T
    return full


def kernel(embed1: np.ndarray, embed2: np.ndarray) -> np.ndarray:
    from concourse.bass_utils import run_bass_kernel_spmd

    nc = _get_nc()
    in_maps = make_in_maps(np.asarray(embed1), np.asarray(embed2))
    res = run_bass_kernel_spmd(nc, in_maps, core_ids=list(range(N_CORES)))
    return assemble(res.results)


if __name__ == "__main__":
    e1 = np.random.randn(N, D).astype(np.float32)
    e2 = np.random.randn(N, D).astype(np.float32)
    out = kernel(embed1=e1, embed2=e2)
    ref = np.max(np.abs(e1[:, None, :] - e2[None, :, :]), axis=2)
    err = np.abs(out - ref).max() / np.abs(ref).max()
    print("rel err:", err)



# revision 3
# speedup vs baseline: 1.0237x; 1.0237x over previous
"""Chebyshev (L-inf) pairwise distance matrix on 8 TRN2 NeuronCores, v2.

reference: out[i, j] = max_d |embed1[i, d] - embed2[j, d]|
  embed1: [4096, 32] f32, embed2: [4096, 32] f32, out: [4096, 4096] f32

v2 strategy: a custom DVE op CHEB_CHAIN_ANT computing
    z' = max(|in0 - s0|, in1)
(ABSOLUTE_DIFF + MAX, 2 ALU stages) with a hand-authored 2x_1P uop
program (bf16 packed pairs, mirroring the stock TENSOR_TENSOR slot-9
wiring), registered at runtime into concourse's custom-DVE table
machinery. One chain op per (j-block, d) fuses absdiff + running max,
so the whole kernel is 32 DVE ops per j-block with no intermediate
absdiff tiles and no ACT dependency. perf_max is set via byte-36[7:6]
(row | 0x40).

Sharding: 8 cores = 2 i-halves x 4 j-quarters. Core computes the
[1024 j, 2048 i] transposed block: partition axis = j (8 blocks of
128), free axis = i (2048).
"""

import sys

if "/opt/trn_rl_repo" not in sys.path:
    sys.path.insert(0, "/opt/trn_rl_repo")

from contextlib import ExitStack

import ml_dtypes
import numpy as np

import concourse.bacc as bacc
import concourse.bass as bass  # noqa: F401
import concourse.tile as tile
from concourse import mybir

BF16 = ml_dtypes.bfloat16

N = 4096
D = 32
N_CORES = 8
N_IQ = 2          # i split (embed1 rows)
N_JH = 4          # j split (embed2 rows)
I_PER = N // N_IQ       # 2048
J_PER = N // N_JH       # 1024
JB = J_PER // 128       # 8 j-blocks per core
U = I_PER               # free-dim elements per d-slot

# e1r slab arrives in chunks so the first chain ops don't wait on 16 MB.
CHUNKS = (1, 1, 2, 2, 4, 4, 6, 6, 6)
assert sum(CHUNKS) == D

# ---------------------------------------------------------------------------
# Custom DVE op registration (done once at import).
# ---------------------------------------------------------------------------

_CHEB = None  # (DveOp, row)


def _register_cheb_chain():
    global _CHEB
    if _CHEB is not None:
        return _CHEB

    from concourse import dve_ops
    from concourse.dve_spec import Spec, Src0, Src1, C0, Bin, lower
    from concourse.dve_spec import AluOp as SAluOp
    from concourse.dve_spec import maxx
    from concourse.dve_uop import (
        AluInp,
        AluOp,
        DelayInp,
        DveOpSpec,
        InpSel,
        OutPath,
        OutSel,
        Trigger,
        UopConfig,
    )

    name = "CHEB_CHAIN_ANT"

    def _ref(in0, in1, s0, s1, imm2):
        return np.maximum(np.abs(in0.astype(np.float32) - s0), in1).astype(
            np.float32
        )

    spec = Spec(
        body=maxx(Bin(SAluOp.ABSOLUTE_DIFF, Src0, C0), Src1),
        reference=_ref,
    )

    # --- hand-authored 2x_1P program (mirrors stock TT slot 9 wiring) ---
    # lanes: 0=SRC_0 1=CONST_0 2=SRC_1 3=SRC_0_HI 4=CONST_0 5=SRC_1_HI
    # blk0: absdiff lo   blk1: max lo -> chain0
    # blk2: absdiff hi   blk3: max hi -> rides ALU pipe to blk7
    u = UopConfig()
    u.enable_input(InpSel.SRC_0, 0)
    u.enable_input(InpSel.CONST_0, 1)
    u.enable_input(InpSel.SRC_1, 2)
    u.enable_input(InpSel.SRC_0_HI, 3)
    u.enable_input(InpSel.CONST_0, 4)
    u.enable_input(InpSel.SRC_1_HI, 5)
    u.require_inp0 = 1
    u.require_inp1 = 1
    u.trigger = (Trigger.SRC_TENSOR_DONE, Trigger.NONE, Trigger.NONE)
    dp = u.datapath_config
    dp[0].enable_alu(AluOp.ABSOLUTE_DIFF, AluInp.PREV_ALU_OUT, AluInp.PREV_DELAY_0)
    dp[0].pass_through_delay(1, 2, 3, 4)
    dp[1].enable_alu(AluOp.MAX, AluInp.PREV_ALU_OUT, AluInp.PREV_DELAY_1)
    dp[1].pass_through_delay(2, 3, 4)
    dp[2].enable_alu(AluOp.ABSOLUTE_DIFF, AluInp.PREV_DELAY_2, AluInp.PREV_DELAY_3)
    dp[2].enable_delay_from_src(DelayInp.PREV_ALU_OUT, 0)
    dp[2].pass_through_delay(4)
    dp[3].enable_alu(AluOp.MAX, AluInp.PREV_ALU_OUT, AluInp.PREV_DELAY_4)
    dp[3].pass_through_delay(0)
    for k in range(4, 8):
        dp[k].pass_through_alu()
        dp[k].pass_through_delay(0)
    u.enable_output(OutSel.DELAY_0, OutPath.WR0_LO)
    u.enable_output(OutSel.ALU_OUT, OutPath.WR0_HI)

    op = dve_ops.DveOp(name, spec, subdim=False, uops_sha={})
    dve_ops.OPS.append(op)
    dve_ops.CUSTOM_DVE_SPECS[name] = spec
    row = max(dve_ops._SUB_OPCODE_FOR_NAME.values()) + 1
    assert row < 0x20
    dve_ops._SUB_OPCODE_FOR_NAME[name] = row

    compiled = DveOpSpec(
        name=name,
        opcode=row,
        uops=lower(spec, ver="v3"),
        uops_2x=[u],
        rd1_en=True,
        perf_max=1,
    )
    # compile() consults this cache first, so the sha pin is bypassed and
    # the 2x program rides along into dve_table_for_ops.
    dve_ops._COMPILE_CACHE[(name, "v3")] = compiled

    _CHEB = (op, row)
    return _CHEB


def _emit_chain(v, op, out, in0, in1, s0, perf_max=1):
    """Emit CHEB_CHAIN_ANT with byte-36[7:6]=perf_max (mirrors
    bass.Vector._custom_dve, minus the paths our op doesn't use)."""
    from concourse import bass_isa
    from concourse.dve_ops import get_dve_sub_opcode

    if op.name not in v.bass.m.ant_custom_dve_ops:
        v.bass.m.ant_custom_dve_ops = sorted(
            {*v.bass.m.ant_custom_dve_ops, op.name}
        )
    shape = bass_isa.CustomDveShape.TTSS
    isa_opcode = v.bass.isa.Opcode[
        f"NEURON_ISA_TPB_OPCODE_CUSTOM_DVE_ANT_{shape.slot()}"
    ].value
    ins = [
        v.lower_ap(in0, for_isa=True, opt=True),
        v.lower_ap(in1, for_isa=True, opt=True),
        v.lower_ap(s0, for_isa=True),
        mybir.ImmediateValue(dtype=mybir.dt.float32, value=0.0),
    ]
    outs = [v.lower_ap(out, for_isa=True, opt=True)]
    return v.add_instruction(
        bass_isa.InstCustomDveAnt(
            name=v.bass.get_next_instruction_name(),
            op_name=op.name,
            rd1_en=True,
            subdim=0,
            imm2=0.0,
            shape=shape,
            row=get_dve_sub_opcode(op.name),
            perf_max=perf_max,
            isa_opcode=isa_opcode,
            ins=ins,
            outs=outs,
        )
    )


# ---------------------------------------------------------------------------
# Kernel build
# ---------------------------------------------------------------------------

_nc_cache = None


def _build_nc():
    op, _row = _register_cheb_chain()

    nc = bacc.Bacc(
        trn_type="TRN2",
        target_bir_lowering=False,
        debug=False,
        num_devices=N_CORES,
    )

    dt_bf16 = mybir.dt.bfloat16
    dt_f32 = mybir.dt.float32

    e1r = nc.declare_dram_parameter("e1r", [128, D * U], dt_bf16, isOutput=False)
    e2b = nc.declare_dram_parameter("e2b", [J_PER, D], dt_f32, isOutput=False)
    out = nc.declare_dram_parameter("out", [J_PER, I_PER], dt_bf16, isOutput=True)

    with tile.TileContext(nc) as tc, ExitStack() as ctx:
        p_e1 = ctx.enter_context(tc.tile_pool(name="e1", bufs=1))
        p_e2 = ctx.enter_context(tc.tile_pool(name="e2", bufs=1))
        p_z = ctx.enter_context(tc.tile_pool(name="z", bufs=2))
        p_out = ctx.enter_context(tc.tile_pool(name="out", bufs=2))

        t_e2 = p_e2.tile([128, JB * D], dt_f32, tag="e2")
        e2_src = e2b[:, :].rearrange("(jb p) d -> p jb d", p=128)
        nc.sync.dma_start(t_e2[:].rearrange("p (jb d) -> p jb d", d=D), e2_src)

        t_zero = p_e2.tile([128, U], dt_bf16, tag="zero")
        nc.vector.memset(t_zero[:], 0.0)

        # e1 slab in chunks, smallest first
        slot_tile = {}
        off = 0
        for ci, csz in enumerate(CHUNKS):
            t = p_e1.tile([128, csz * U], dt_bf16, tag=f"e1c{ci}")
            nc.sync.dma_start(t[:], e1r[:, off * U:(off + csz) * U])
            for kk in range(csz):
                slot_tile[off + kk] = (t, kk)
            off += csz

        for jb in range(JB):
            t_z = p_z.tile([128, U], dt_bf16, tag="z")
            t_out = p_out.tile([128, U], dt_bf16, tag="out")
            for k in range(D):
                t, kk = slot_tile[k]
                in0 = t[:, kk * U:(kk + 1) * U]
                s0 = t_e2[:, jb * D + k: jb * D + k + 1]
                in1 = t_zero[:] if k == 0 else t_z[:]
                dst = t_out[:] if k == D - 1 else t_z[:]
                _emit_chain(nc.vector, op, out=dst, in0=in0, in1=in1, s0=s0)
            nc.sync.dma_start(out[jb * 128:(jb + 1) * 128, :], t_out[:])

    nc.finalize()
    return nc


def _get_nc():
    global _nc_cache
    if _nc_cache is None:
        _nc_cache = _build_nc()
    return _nc_cache


def make_in_maps(embed1: np.ndarray, embed2: np.ndarray):
    """Core c: iq = c % N_IQ, jh = c // N_IQ."""
    embed1 = np.asarray(embed1, dtype=np.float32)
    embed2 = np.asarray(embed2, dtype=np.float32)
    in_maps = []
    for c in range(N_CORES):
        iq, jh = c % N_IQ, c // N_IQ
        e1_slab = embed1[iq * I_PER:(iq + 1) * I_PER, :]      # [2048, 32]
        flat = np.ascontiguousarray(e1_slab.T).reshape(-1).astype(BF16)
        rep = np.ascontiguousarray(
            np.broadcast_to(flat[None, :], (128, D * I_PER))
        )
        e2_slab = np.ascontiguousarray(
            embed2[jh * J_PER:(jh + 1) * J_PER, :]
        )  # [1024, 32]
        in_maps.append({"e1r": rep, "e2b": e2_slab})
    return in_maps


def assemble(results) -> np.ndarray:
    full = np.empty((N, N), dtype=np.float32)
    for c in range(N_CORES):
        iq, jh = c % N_IQ, c // N_IQ
        blk = np.asarray(results[c]["out"]).astype(np.float32)  # [1024, 2048]
        full[iq * I_PER:(iq + 1) * I_PER, jh * J_PER:(jh + 1) * J_PER] = blk.T
    return full


def kernel(embed1: np.ndarray, embed2: np.ndarray) -> np.ndarray:
    from concourse.bass_utils import run_bass_kernel_spmd

    nc = _get_nc()
    in_maps = make_in_maps(np.asarray(embed1), np.asarray(embed2))
    res = run_bass_kernel_spmd(nc, in_maps, core_ids=list(range(N_CORES)))
    return assemble(res.results)


if __name__ == "__main__":
    e1 = np.random.randn(N, D).astype(np.float32)
    e2 = np.random.randn(N, D).astype(np.float32)
    out = kernel(embed1=e1, embed2=e2)
    ref = np.max(np.abs(e1[:, None, :] - e2[None, :, :]), axis=2)
    err = np.abs(out - ref).max() / np.abs(ref).max()
    print("rel err:", err)


# revision 4
# speedup vs baseline: 1.0242x; 1.0005x over previous
"""Chebyshev (L-inf) pairwise distance matrix on 8 TRN2 NeuronCores, v2.

reference: out[i, j] = max_d |embed1[i, d] - embed2[j, d]|
  embed1: [4096, 32] f32, embed2: [4096, 32] f32, out: [4096, 4096] f32

v2 strategy: a custom DVE op CHEB_CHAIN_ANT computing
    z' = max(|in0 - s0|, in1)
(ABSOLUTE_DIFF + MAX, 2 ALU stages) with a hand-authored 2x_1P uop
program (bf16 packed pairs, mirroring the stock TENSOR_TENSOR slot-9
wiring), registered at runtime into concourse's custom-DVE table
machinery. One chain op per (j-block, d) fuses absdiff + running max,
so the whole kernel is 32 DVE ops per j-block with no intermediate
absdiff tiles and no ACT dependency. perf_max is set via byte-36[7:6]
(row | 0x40).

Sharding: 8 cores = 2 i-halves x 4 j-quarters. Core computes the
[1024 j, 2048 i] transposed block: partition axis = j (8 blocks of
128), free axis = i (2048).
"""

import sys

if "/opt/trn_rl_repo" not in sys.path:
    sys.path.insert(0, "/opt/trn_rl_repo")

from contextlib import ExitStack

import ml_dtypes
import numpy as np

import concourse.bacc as bacc
import concourse.bass as bass  # noqa: F401
import concourse.tile as tile
from concourse import mybir

BF16 = ml_dtypes.bfloat16

N = 4096
D = 32
N_CORES = 8
N_IQ = 2          # i split (embed1 rows)
N_JH = 4          # j split (embed2 rows)
I_PER = N // N_IQ       # 2048
J_PER = N // N_JH       # 1024
JB = J_PER // 128       # 8 j-blocks per core
U = I_PER               # free-dim elements per d-slot

# e1r slab arrives in chunks so the first chain ops don't wait on 16 MB.
CHUNKS = (1,) * 8 + (2,) * 12
assert sum(CHUNKS) == D

# ---------------------------------------------------------------------------
# Custom DVE op registration (done once at import).
# ---------------------------------------------------------------------------

_CHEB = None  # (DveOp, row)


def _register_cheb_chain():
    global _CHEB
    if _CHEB is not None:
        return _CHEB

    from concourse import dve_ops
    from concourse.dve_spec import Spec, Src0, Src1, C0, Bin, lower
    from concourse.dve_spec import AluOp as SAluOp
    from concourse.dve_spec import maxx
    from concourse.dve_uop import (
        AluInp,
        AluOp,
        DelayInp,
        DveOpSpec,
        InpSel,
        OutPath,
        OutSel,
        Trigger,
        UopConfig,
    )

    name = "CHEB_CHAIN_ANT"

    def _ref(in0, in1, s0, s1, imm2):
        return np.maximum(np.abs(in0.astype(np.float32) - s0), in1).astype(
            np.float32
        )

    spec = Spec(
        body=maxx(Bin(SAluOp.ABSOLUTE_DIFF, Src0, C0), Src1),
        reference=_ref,
    )

    # --- hand-authored 2x_1P program (mirrors stock TT slot 9 wiring) ---
    # lanes: 0=SRC_0 1=CONST_0 2=SRC_1 3=SRC_0_HI 4=CONST_0 5=SRC_1_HI
    # blk0: absdiff lo   blk1: max lo -> chain0
    # blk2: absdiff hi   blk3: max hi -> rides ALU pipe to blk7
    u = UopConfig()
    u.enable_input(InpSel.SRC_0, 0)
    u.enable_input(InpSel.CONST_0, 1)
    u.enable_input(InpSel.SRC_1, 2)
    u.enable_input(InpSel.SRC_0_HI, 3)
    u.enable_input(InpSel.CONST_0, 4)
    u.enable_input(InpSel.SRC_1_HI, 5)
    u.require_inp0 = 1
    u.require_inp1 = 1
    u.trigger = (Trigger.SRC_TENSOR_DONE, Trigger.NONE, Trigger.NONE)
    dp = u.datapath_config
    dp[0].enable_alu(AluOp.ABSOLUTE_DIFF, AluInp.PREV_ALU_OUT, AluInp.PREV_DELAY_0)
    dp[0].pass_through_delay(1, 2, 3, 4)
    dp[1].enable_alu(AluOp.MAX, AluInp.PREV_ALU_OUT, AluInp.PREV_DELAY_1)
    dp[1].pass_through_delay(2, 3, 4)
    dp[2].enable_alu(AluOp.ABSOLUTE_DIFF, AluInp.PREV_DELAY_2, AluInp.PREV_DELAY_3)
    dp[2].enable_delay_from_src(DelayInp.PREV_ALU_OUT, 0)
    dp[2].pass_through_delay(4)
    dp[3].enable_alu(AluOp.MAX, AluInp.PREV_ALU_OUT, AluInp.PREV_DELAY_4)
    dp[3].pass_through_delay(0)
    for k in range(4, 8):
        dp[k].pass_through_alu()
        dp[k].pass_through_delay(0)
    u.enable_output(OutSel.DELAY_0, OutPath.WR0_LO)
    u.enable_output(OutSel.ALU_OUT, OutPath.WR0_HI)

    op = dve_ops.DveOp(name, spec, subdim=False, uops_sha={})
    dve_ops.OPS.append(op)
    dve_ops.CUSTOM_DVE_SPECS[name] = spec
    row = max(dve_ops._SUB_OPCODE_FOR_NAME.values()) + 1
    assert row < 0x20
    dve_ops._SUB_OPCODE_FOR_NAME[name] = row

    compiled = DveOpSpec(
        name=name,
        opcode=row,
        uops=lower(spec, ver="v3"),
        uops_2x=[u],
        rd1_en=True,
        perf_max=1,
    )
    # compile() consults this cache first, so the sha pin is bypassed and
    # the 2x program rides along into dve_table_for_ops.
    dve_ops._COMPILE_CACHE[(name, "v3")] = compiled

    _CHEB = (op, row)
    return _CHEB


def _emit_chain(v, op, out, in0, in1, s0, perf_max=1):
    """Emit CHEB_CHAIN_ANT with byte-36[7:6]=perf_max (mirrors
    bass.Vector._custom_dve, minus the paths our op doesn't use)."""
    from concourse import bass_isa
    from concourse.dve_ops import get_dve_sub_opcode

    if op.name not in v.bass.m.ant_custom_dve_ops:
        v.bass.m.ant_custom_dve_ops = sorted(
            {*v.bass.m.ant_custom_dve_ops, op.name}
        )
    shape = bass_isa.CustomDveShape.TTSS
    isa_opcode = v.bass.isa.Opcode[
        f"NEURON_ISA_TPB_OPCODE_CUSTOM_DVE_ANT_{shape.slot()}"
    ].value
    ins = [
        v.lower_ap(in0, for_isa=True, opt=True),
        v.lower_ap(in1, for_isa=True, opt=True),
        v.lower_ap(s0, for_isa=True),
        mybir.ImmediateValue(dtype=mybir.dt.float32, value=0.0),
    ]
    outs = [v.lower_ap(out, for_isa=True, opt=True)]
    return v.add_instruction(
        bass_isa.InstCustomDveAnt(
            name=v.bass.get_next_instruction_name(),
            op_name=op.name,
            rd1_en=True,
            subdim=0,
            imm2=0.0,
            shape=shape,
            row=get_dve_sub_opcode(op.name),
            perf_max=perf_max,
            isa_opcode=isa_opcode,
            ins=ins,
            outs=outs,
        )
    )


# ---------------------------------------------------------------------------
# Kernel build
# ---------------------------------------------------------------------------

_nc_cache = None


def _build_nc():
    op, _row = _register_cheb_chain()

    nc = bacc.Bacc(
        trn_type="TRN2",
        target_bir_lowering=False,
        debug=False,
        num_devices=N_CORES,
    )

    dt_bf16 = mybir.dt.bfloat16
    dt_f32 = mybir.dt.float32

    e1r = nc.declare_dram_parameter("e1r", [128, D * U], dt_bf16, isOutput=False)
    e2b = nc.declare_dram_parameter("e2b", [J_PER, D], dt_f32, isOutput=False)
    out = nc.declare_dram_parameter("out", [J_PER, I_PER], dt_bf16, isOutput=True)

    with tile.TileContext(nc) as tc, ExitStack() as ctx:
        p_e1 = ctx.enter_context(tc.tile_pool(name="e1", bufs=1))
        p_e2 = ctx.enter_context(tc.tile_pool(name="e2", bufs=1))
        p_z = ctx.enter_context(tc.tile_pool(name="z", bufs=2))
        p_out = ctx.enter_context(tc.tile_pool(name="out", bufs=2))

        t_e2 = p_e2.tile([128, JB * D], dt_f32, tag="e2")
        e2_src = e2b[:, :].rearrange("(jb p) d -> p jb d", p=128)
        nc.sync.dma_start(t_e2[:].rearrange("p (jb d) -> p jb d", d=D), e2_src)

        t_zero = p_e2.tile([128, U], dt_bf16, tag="zero")
        nc.vector.memset(t_zero[:], 0.0)

        # e1 slab in chunks, smallest first
        slot_tile = {}
        off = 0
        for ci, csz in enumerate(CHUNKS):
            t = p_e1.tile([128, csz * U], dt_bf16, tag=f"e1c{ci}")
            nc.sync.dma_start(t[:], e1r[:, off * U:(off + csz) * U])
            for kk in range(csz):
                slot_tile[off + kk] = (t, kk)
            off += csz

        for jb in range(JB):
            t_z = p_z.tile([128, U], dt_bf16, tag="z")
            t_out = p_out.tile([128, U], dt_bf16, tag="out")
            for k in range(D):
                t, kk = slot_tile[k]
                in0 = t[:, kk * U:(kk + 1) * U]
                s0 = t_e2[:, jb * D + k: jb * D + k + 1]
                in1 = t_zero[:] if k == 0 else t_z[:]
                dst = t_out[:] if k == D - 1 else t_z[:]
                _emit_chain(nc.vector, op, out=dst, in0=in0, in1=in1, s0=s0)
            nc.sync.dma_start(out[jb * 128:(jb + 1) * 128, :], t_out[:])

    nc.finalize()
    return nc


def _get_nc():
    global _nc_cache
    if _nc_cache is None:
        _nc_cache = _build_nc()
    return _nc_cache


def make_in_maps(embed1: np.ndarray, embed2: np.ndarray):
    """Core c: iq = c % N_IQ, jh = c // N_IQ."""
    embed1 = np.asarray(embed1, dtype=np.float32)
    embed2 = np.asarray(embed2, dtype=np.float32)
    in_maps = []
    for c in range(N_CORES):
        iq, jh = c % N_IQ, c // N_IQ
        e1_slab = embed1[iq * I_PER:(iq + 1) * I_PER, :]      # [2048, 32]
        flat = np.ascontiguousarray(e1_slab.T).reshape(-1).astype(BF16)
        rep = np.ascontiguousarray(
            np.broadcast_to(flat[None, :], (128, D * I_PER))
        )
        e2_slab = np.ascontiguousarray(
            embed2[jh * J_PER:(jh + 1) * J_PER, :]
        )  # [1024, 32]
        in_maps.append({"e1r": rep, "e2b": e2_slab})
    return in_maps


def assemble(results) -> np.ndarray:
    full = np.empty((N, N), dtype=np.float32)
    for c in range(N_CORES):
        iq, jh = c % N_IQ, c // N_IQ
        blk = np.asarray(results[c]["out"]).astype(np.float32)  # [1024, 2048]
        full[iq * I_PER:(iq + 1) * I_PER, jh * J_PER:(jh + 1) * J_PER] = blk.T
    return full


def kernel(embed1: np.ndarray, embed2: np.ndarray) -> np.ndarray:
    from concourse.bass_utils import run_bass_kernel_spmd

    nc = _get_nc()
    in_maps = make_in_maps(np.asarray(embed1), np.asarray(embed2))
    res = run_bass_kernel_spmd(nc, in_maps, core_ids=list(range(N_CORES)))
    return assemble(res.results)


if __name__ == "__main__":
    e1 = np.random.randn(N, D).astype(np.float32)
    e2 = np.random.randn(N, D).astype(np.float32)
    out = kernel(embed1=e1, embed2=e2)
    ref = np.max(np.abs(e1[:, None, :] - e2[None, :, :]), axis=2)
    err = np.abs(out - ref).max() / np.abs(ref).max()
    print("rel err:", err)
